# revision 22
# baseline (speedup 1.0000x reference)
"""Trainium2 Bass kernel for nn_CINComp: out[b,o,d] = sum_{i,j} W[o,i*64+j]*feature[b,i,d]*base[b,j,d] + bias[o].

Sharding: data-parallel over batch B=1024 across 8 cores (128 batches/core).

Per-core algorithm (v3, engine-balanced hybrid):
  - contraction dim ij = 4096 split into 32 K-chunks of 128 = (2 i-rows x 64 j)
  - gt2[p=(dup,j), (b,d)] holds G transposed + duplicated (host-prepped, bf16)
  - the F-row broadcast for chunks 0..25 (fbc) is PRE-BUILT ON HOST in bf16
    and streamed from HBM; for chunks 26..31 it is produced ON CHIP by K=16
    selector matmuls (PE) + ACT copies PSUM->SBUF-bf16, trading DMA bytes for
    idle PE/ACT cycles so PE, DMA, DVE and ACT all land ~8.4us/group
  - every multiply is an all-SBUF bf16 DVE tensor_tensor -> 2x_1P mode (the
    original baseline read PSUM fp32 -> 1x mode = 2x slower, its bottleneck)
  - PE contracts W^T-chunk @ P into PSUM acc (bf16, FWL, LDW hidden in MMs)
  - ACT adds bias during the PSUM->SBUF output copy (Identity w/ bias AP)

Sync discipline: fused LDWEIGHTS+MATMUL, TT, ACT and DMA instruction structs
accept only ONE semaphore wait.  Every DMA-landed tile is "touched" with a
1-element self-copy on the engine whose FIFO must carry the dependency, and
cross-engine completion is chained (ACT flag tile -> DVE read) so that every
compute instruction needs at most one wait; _strip_self_waits then drops all
transitively-implied waits.
"""

import numpy as np

import concourse.bass as bass
import concourse.mybir as mybir
import concourse.tile as tile
from concourse.bass import ts
from concourse.bass_utils import run_bass_kernel_spmd

B, HK, H0, D, O = 1024, 64, 64, 32, 128
NCORES = 8
BLOC = B // NCORES          # 128 batches per core
GROUPS = 8                  # batch groups per core
GB = BLOC // GROUPS         # 16 batches per group
N = GB * D                  # 512 = matmul free dim per group
NCHUNK = 32                 # K chunks of 128 over ij=4096
NSEL = 4                    # chunks 28..31 produced on-chip via PE selectors
NFBC = NCHUNK - NSEL        # 28 chunks streamed from HBM
F32 = mybir.dt.float32
BF16 = mybir.dt.bfloat16

_CACHE = {}


def _sel_const() -> np.ndarray:
    # sel[k, cs, m] = 1 iff k == 2*cs + (m // 64): K=16 selector that
    # broadcasts compact-ft row 2*cs+delta (= F row 52+2*cs+delta) to
    # partition half delta for on-chip chunk NFBC+cs.
    sel = np.zeros((16, NSEL, 128), np.float32)
    for cs in range(NSEL):
        for m in range(128):
            sel[2 * cs + (m // 64), cs, m] = 1.0
    return sel.reshape(16, NSEL * 128)


def _strip_self_waits(nc: bass.Bass) -> None:
    """Transitively-minimal semaphore waits (see module docstring)."""
    UPD = ("sem-inc", "sem-add-imm")
    insts = [i for bb in nc.m.functions[0].blocks for i in bb.instructions]

    bad_sems = set()
    for i in insts:
        si = getattr(i, "sync_info", None)
        if si is None:
            continue
        for u in si.on_update:
            if u.sync_type != "semaphore" or u.update_mode not in UPD:
                bad_sems.add(u.id)

    def fifo_of(i):
        si = i.sync_info
        eng = str(getattr(i, "engine", None))
        if type(i).__name__ == "InstDMACopy" and si is not None:
            for u in si.on_update:
                if u.sync_type == "semaphore" and u.update_mode in UPD:
                    return ("q", u.id)
        return ("e", eng)

    cum: dict = {}
    event: dict = {}
    fifo_pred: dict = {}
    last_in_fifo: dict = {}
    metas = []
    for idx, i in enumerate(insts):
        si = getattr(i, "sync_info", None)
        f = fifo_of(i)
        fifo_pred[idx] = last_in_fifo.get(f)
        last_in_fifo[f] = idx
        ups = []
        if si is not None:
            for u in si.on_update:
                if u.sync_type == "semaphore" and u.update_mode in UPD:
                    cum[u.id] = cum.get(u.id, 0) + u.update_value
                    event[(u.id, cum[u.id])] = idx
                    ups.append((u.id, cum[u.id]))
        metas.append((si, ups))

    def resolve(sem, k):
        v = k
        while (sem, v) not in event:
            v += 1
            if v > cum.get(sem, 0):
                return None
        return event[(sem, v)]

    cvc: list = [None] * len(insts)

    def get_cvc(idx):
        if cvc[idx] is not None:
            return cvc[idx]
        stack = [idx]
        while stack:
            j = stack[-1]
            if cvc[j] is not None:
                stack.pop()
                continue
            si, ups = metas[j]
            deps = []
            p = fifo_pred[j]
            if p is not None:
                deps.append(p)
            if si is not None:
                for w in si.on_wait:
                    if (
                        w.sync_type == "semaphore"
                        and w.wait_mode == "sem-ge-imm"
                        and w.id not in bad_sems
                    ):
                        e = resolve(w.id, w.wait_value)
                        if e is not None and e != j:
                            deps.append(e)
            pending = [d for d in deps if cvc[d] is None]
            if pending:
                stack.extend(pending)
                continue
            stack.pop()
            vc: dict = {}
            for d in deps:
                for s, v in cvc[d].items():
                    if vc.get(s, 0) < v:
                        vc[s] = v
            if si is not None:
                for w in si.on_wait:
                    if (
                        w.sync_type == "semaphore"
                        and w.wait_mode == "sem-ge-imm"
                        and w.id not in bad_sems
                    ):
                        if vc.get(w.id, 0) < w.wait_value:
                            vc[w.id] = w.wait_value
            for s, v in ups:
                if vc.get(s, 0) < v:
                    vc[s] = v
            cvc[j] = vc
        return cvc[idx]

    for idx, i in enumerate(insts):
        si, _ups = metas[idx]
        if si is None or not si.on_wait:
            continue
        base: dict = {}
        p = fifo_pred[idx]
        if p is not None:
            base = dict(get_cvc(p))
        sem_waits = [
            w
            for w in si.on_wait
            if w.sync_type == "semaphore"
            and w.wait_mode == "sem-ge-imm"
            and w.id not in bad_sems
        ]
        other = [w for w in si.on_wait if w not in sem_waits]

        def strength(w):
            e = resolve(w.id, w.wait_value)
            return len(get_cvc(e)) if e is not None else 0

        sem_waits.sort(key=strength, reverse=True)

        def wait_cvc(w):
            e = resolve(w.id, w.wait_value)
            vc = dict(get_cvc(e)) if e is not None else {}
            if vc.get(w.id, 0) < w.wait_value:
                vc[w.id] = w.wait_value
            return vc

        kept = sem_waits[:]
        changed = True
        while changed:
            changed = False
            for w in kept:
                cover = dict(base)
                for w2 in kept:
                    if w2 is w:
                        continue
                    for s, v in wait_cvc(w2).items():
                        if cover.get(s, 0) < v:
                            cover[s] = v
                if cover.get(w.id, 0) >= w.wait_value:
                    kept.remove(w)
                    changed = True
                    break
        if len(kept) + len(other) != len(si.on_wait):
            si.on_wait = other + kept


def _segs(g: int) -> list:
    # fbc sub-DMA / TT granularity per group: fine for the ramp group, big
    # (fewer DVE ops) mid-stream, fine again to shorten the pipeline tail
    if g == 0:
        return [7, 7, 7, 7]
    if g == GROUPS - 1:
        return [14, 7, 7]
    return [14, 14]


def _build_nc(strip: bool = True) -> bass.Bass:
    nc = bass.Bass()
    wt = nc.dram_tensor("wt", [128, NCHUNK * 128], BF16, kind="ExternalInput")
    gt2 = nc.dram_tensor("gt2", [128, BLOC * D], BF16, kind="ExternalInput")
    fbc = nc.dram_tensor("fbc", [128, GROUPS * NFBC * N], BF16,
                         kind="ExternalInput")
    ft = nc.dram_tensor("ft", [16, BLOC * D], BF16, kind="ExternalInput")
    sel = nc.dram_tensor("sel", [16, NSEL * 128], BF16, kind="ExternalInput")
    bias = nc.dram_tensor("bias", [128, 1], F32, kind="ExternalInput")
    out = nc.dram_tensor("out", [128, BLOC * D], BF16, kind="ExternalOutput")

    GSZ = NFBC * N              # fbc elems per group per partition
    AF = mybir.ActivationFunctionType

    with tile.TileContext(nc) as tc:
        with (
            tc.tile_pool(name="resident", bufs=1) as res,
            tc.tile_pool(name="fbcp", bufs=2) as fpool,
            tc.tile_pool(name="p", bufs=2) as ppool,
            tc.tile_pool(name="fsbp", bufs=2 * GROUPS) as spool,
            tc.tile_pool(name="osb", bufs=8) as opool,
            tc.tile_pool(name="acc", bufs=4, space="PSUM") as apool,
            tc.tile_pool(name="fps", bufs=2, space="PSUM") as qpool,
        ):
            gt2_sb = res.tile([128, BLOC * D], BF16)
            wt_sb = res.tile([128, NCHUNK * 128], BF16)
            ft_sb = res.tile([16, BLOC * D], BF16)
            sel_sb = res.tile([16, NSEL * 128], BF16)
            bias_sb = res.tile([128, 1], F32)
            flag = res.tile([128, 4], BF16)
            scr = res.tile([128, 1], BF16)

            # scalar-HWDGE ring: small/bulky resident inputs + outputs.
            # sync ring: group-0 gt2 slice, then the fbc stream (so the fbc
            # flood cannot delay the inputs' transfers and vice versa).
            nc.scalar.dma_start(out=bias_sb[:], in_=bias[:])
            nc.scalar.dma_start(out=ft_sb[:], in_=ft[:])
            nc.scalar.dma_start(out=sel_sb[:], in_=sel[:])
            nc.scalar.dma_start(out=wt_sb[:], in_=wt[:])
            nc.scalar.dma_start(out=gt2_sb[:, N:], in_=gt2[:, N:])
            nc.scalar.activation(bias_sb[0:1, 0:1], bias_sb[0:1, 0:1], AF.Copy)
            nc.sync.dma_start(out=gt2_sb[:, ts(0, N)], in_=gt2[:, ts(0, N)])
            nc.vector.tensor_copy(gt2_sb[0:1, 0:1], gt2_sb[0:1, 0:1])
            nc.vector.tensor_copy(ft_sb[0:1, 0:1], ft_sb[0:1, 0:1])
            nc.vector.tensor_copy(sel_sb[0:1, 0:1], sel_sb[0:1, 0:1])
            nc.vector.tensor_copy(wt_sb[0:1, 0:1], wt_sb[0:1, 0:1])
            nc.vector.tensor_copy(gt2_sb[0:1, N:N + 1], gt2_sb[0:1, N:N + 1])

            fbc_tiles = {}

            def issue_fbc(g):
                t = fpool.tile([128, NFBC, N], BF16, tag="fbc")
                fbc_tiles[g] = t
                c0 = 0
                for sg in _segs(g):
                    nc.sync.dma_start(
                        out=t[:, c0:c0 + sg, :],
                        in_=fbc[:, g * GSZ + c0 * N:g * GSZ + (c0 + sg) * N],
                    )
                    c0 += sg

            def sel_mms(g):
                # selector matmuls producing group g's on-chip broadcast
                # pairs (chunks NFBC..31) into PSUM, K=16 compact contraction
                tiles = []
                for u in range(NSEL // 2):
                    fp = qpool.tile([128, 2, N], F32, tag="fps")
                    for d_ in range(2):
                        cs = 2 * u + d_
                        nc.tensor.matmul(fp[:, d_, :],
                                         sel_sb[:, ts(cs, 128)],
                                         ft_sb[:, ts(g, N)],
                                         start=True, stop=True)
                    tiles.append(fp)
                return tiles

            def act_copies(fps_tiles, g):
                # fsb pool has one buffer per (group, pair): never reused, so
                # these ACT copies carry no WAR wait (1-wait struct limit)
                tiles = []
                for u, fp in enumerate(fps_tiles):
                    fsb = spool.tile([128, 2, N], BF16, tag="fsb")
                    nc.scalar.activation(fsb[:], fp[:], AF.Copy)
                    tiles.append(fsb)
                return tiles

            issue_fbc(0)
            issue_fbc(1)
            fps_cur = sel_mms(0)
            fsb_cur = act_copies(fps_cur, 0)

            for g in range(GROUPS):
                fbc_t = fbc_tiles.pop(g)
                if g + 2 < GROUPS:
                    issue_fbc(g + 2)
                acc = apool.tile([128, N], F32, tag="acc")
                gblk = gt2_sb[:, ts(g, N)]
                # deferred flag-read: pulls group g-2's ACT completion onto
                # the DVE clock well before group g+2 needs it for acc reuse,
                # without stalling DVE at the g-1/g boundary (identity(g-2)
                # finished long ago).  4 flag slots keep the RAW dep bound to
                # the right flag-copy.
                if g >= 2:
                    s_ = (g - 2) % 4
                    nc.vector.tensor_copy(scr[0:1, 0:1],
                                          flag[0:1, s_:s_ + 1])
                c0 = 0
                for sg in _segs(g):
                    # touch: each sub-DMA's completion sem onto the DVE clock
                    nc.vector.tensor_copy(fbc_t[0:1, c0, 0:1],
                                          fbc_t[0:1, c0, 0:1])
                    p = ppool.tile([128, sg, N], BF16, tag=f"p{sg}", bufs=2)
                    gview = gblk[:, None, :].to_broadcast((128, sg, N))
                    nc.vector.tensor_mul(p[:], gview,
                                         fbc_t[:, c0:c0 + sg, :])
                    for u in range(sg):
                        c = c0 + u
                        nc.tensor.matmul(acc[:], wt_sb[:, ts(c, 128)],
                                         p[:, u, :], start=(c == 0),
                                         stop=False)
                    c0 += sg

                # on-chip chunks NFBC..31: TT from the ACT-copied SBUF tiles.
                # pp tiles are never reused (one per group+pair) -> the TTs
                # carry only their single fsb-RAW (ACT) wait
                for u, fsb in enumerate(fsb_cur):
                    p2 = ppool.tile([128, 2, N], BF16, tag="pp",
                                    bufs=GROUPS * NSEL // 2)
                    gview = gblk[:, None, :].to_broadcast((128, 2, N))
                    nc.vector.tensor_mul(p2[:], gview, fsb[:])
                    for d_ in range(2):
                        c = NFBC + 2 * u + d_
                        nc.tensor.matmul(acc[:], wt_sb[:, ts(c, 128)],
                                         p2[:, d_, :], start=False,
                                         stop=(c == NCHUNK - 1))

                # next group's selector pipeline (PE then ACT), issued at the
                # end of this group's PE stream so WAR chains stay implied
                if g + 1 < GROUPS:
                    fps_cur = sel_mms(g + 1)

                osb = opool.tile([128, N], BF16, tag="osb")
                nc.scalar.activation(osb[:], acc[:], AF.Identity,
                                     bias=bias_sb[:, 0:1], scale=1.0)
                if g + 1 < GROUPS:
                    fsb_cur = act_copies(fps_cur, g + 1)
                # identity(g) -> flag-copy (reads osb!): REAL data deps (the
                # scheduler reorders freely; only data deps pin order); the
                # matching DVE read happens at the start of group g+2
                s_ = g % 4
                nc.scalar.activation(flag[0:1, s_:s_ + 1], osb[0:1, 1:2],
                                     AF.Copy)
                nc.scalar.dma_start(out=out[:, ts(g, N)], in_=osb[:])
                # WAR consumer: pulls the out-DMA's queue-sem onto the ACT
                # clock so the kernel-tail drain needs only engine waits
                nc.scalar.activation(osb[0:1, 0:1], osb[0:1, 0:1], AF.Copy)
                last_osb = osb

            # fold the ACT-final clock onto DVE so the kernel-exit Drain
            # (a 1-wait CTRL struct) needs only the DVE sem
            nc.vector.tensor_copy(scr[0:1, 0:1], last_osb[0:1, 0:1])

    if strip:
        _strip_self_waits(nc)
    return nc


def _get_nc() -> bass.Bass:
    if "nc" not in _CACHE:
        _CACHE["nc"] = _build_nc()
    return _CACHE["nc"]


def _prep_core_inputs(feature, base, W, b, ci):
    bf16 = mybir.dt.np(BF16)
    bsl = slice(ci * BLOC, (ci + 1) * BLOC)
    F = np.ascontiguousarray(feature[bsl], np.float32)  # (128, 64, 32)
    G = np.ascontiguousarray(base[bsl], np.float32)     # (128, 64, 32)

    Gt = np.transpose(G, (1, 0, 2))                      # (j, b, d)
    gt2 = np.concatenate([Gt, Gt], 0).reshape(128, BLOC * D)

    # wt[p, c, o] = W[o, 128c + p]
    wt = np.transpose(W.reshape(O, NCHUNK, 128), (2, 1, 0)).reshape(
        128, NCHUNK * 128)

    # fbc[p=(dup,64), g, c, n] = F-transposed[i=2c+dup, g, n] replicated over
    # the 64 j-partitions of each dup-half; chunks 0..25 only (26..31 are
    # produced on-chip from ft via selector matmuls)
    Ft = np.transpose(F, (1, 0, 2)).reshape(HK, GROUPS, N)   # [i, g, n]
    A = Ft.reshape(NCHUNK, 2, GROUPS, N).astype(bf16)        # [c, dup, g, n]
    fbc = np.broadcast_to(
        A.transpose(1, 2, 0, 3)[:, None, :, :NFBC], (2, 64, GROUPS, NFBC, N)
    )
    fbc = np.ascontiguousarray(fbc).reshape(128, GROUPS * NFBC * N)

    # compact ft rows: F rows 2*NFBC.. at partitions 0.., zero-padded to 16
    nr = 2 * NSEL
    ftc = np.zeros((16, BLOC * D), bf16)
    ftc[:nr] = np.transpose(F[:, 2 * NFBC:, :], (1, 0, 2)).reshape(
        nr, BLOC * D)

    return {
        "wt": np.ascontiguousarray(wt.astype(bf16)),
        "gt2": np.ascontiguousarray(gt2.astype(bf16)),
        "fbc": fbc,
        "ft": ftc,
        "sel": np.ascontiguousarray(_sel_const().astype(bf16)),
        "bias": np.ascontiguousarray(b, np.float32).reshape(128, 1),
    }


def run(feature, base, W, b, **spmd_kwargs):
    nc = _get_nc()
    in_maps = [_prep_core_inputs(feature, base, W, b, ci) for ci in range(NCORES)]
    res = run_bass_kernel_spmd(nc, in_maps, list(range(NCORES)), **spmd_kwargs)
    outs = []
    for ci in range(NCORES):
        o = res.results[ci]["out"].astype(np.float32).reshape(O, BLOC, D)
        outs.append(np.transpose(o, (1, 0, 2)))
    full = np.concatenate(outs, 0)
    return full, res


def kernel(feature, base, W, b):
    full, _ = run(feature, base, W, b)
    return full


# revision 23
# speedup vs baseline: 1.0224x; 1.0224x over previous
"""Trainium2 Bass kernel for nn_CINComp: out[b,o,d] = sum_{i,j} W[o,i*64+j]*feature[b,i,d]*base[b,j,d] + bias[o].

Sharding: data-parallel over batch B=1024 across 8 cores (128 batches/core).

Per-core algorithm (v3, engine-balanced hybrid):
  - contraction dim ij = 4096 split into 32 K-chunks of 128 = (2 i-rows x 64 j)
  - gt2[p=(dup,j), (b,d)] holds G transposed + duplicated (host-prepped, bf16)
  - the F-row broadcast for chunks 0..25 (fbc) is PRE-BUILT ON HOST in bf16
    and streamed from HBM; for chunks 26..31 it is produced ON CHIP by K=16
    selector matmuls (PE) + ACT copies PSUM->SBUF-bf16, trading DMA bytes for
    idle PE/ACT cycles so PE, DMA, DVE and ACT all land ~8.4us/group
  - every multiply is an all-SBUF bf16 DVE tensor_tensor -> 2x_1P mode (the
    original baseline read PSUM fp32 -> 1x mode = 2x slower, its bottleneck)
  - PE contracts W^T-chunk @ P into PSUM acc (bf16, FWL, LDW hidden in MMs)
  - ACT adds bias during the PSUM->SBUF output copy (Identity w/ bias AP)

Sync discipline: fused LDWEIGHTS+MATMUL, TT, ACT and DMA instruction structs
accept only ONE semaphore wait.  Every DMA-landed tile is "touched" with a
1-element self-copy on the engine whose FIFO must carry the dependency, and
cross-engine completion is chained (ACT flag tile -> DVE read) so that every
compute instruction needs at most one wait; _strip_self_waits then drops all
transitively-implied waits.
"""

import numpy as np

import concourse.bass as bass
import concourse.mybir as mybir
import concourse.tile as tile
from concourse.bass import ts
from concourse.bass_utils import run_bass_kernel_spmd

B, HK, H0, D, O = 1024, 64, 64, 32, 128
NCORES = 8
BLOC = B // NCORES          # 128 batches per core
GROUPS = 8                  # batch groups per core
GB = BLOC // GROUPS         # 16 batches per group
N = GB * D                  # 512 = matmul free dim per group
NCHUNK = 32                 # K chunks of 128 over ij=4096
NSEL = 4                    # chunks 28..31 produced on-chip via PE selectors
NFBC = NCHUNK - NSEL        # 28 chunks streamed from HBM
F32 = mybir.dt.float32
BF16 = mybir.dt.bfloat16

_CACHE = {}


def _sel_const() -> np.ndarray:
    # sel[k, cs, m] = 1 iff k == 2*cs + (m // 64): K=16 selector that
    # broadcasts compact-ft row 2*cs+delta (= F row 52+2*cs+delta) to
    # partition half delta for on-chip chunk NFBC+cs.
    sel = np.zeros((16, NSEL, 128), np.float32)
    for cs in range(NSEL):
        for m in range(128):
            sel[2 * cs + (m // 64), cs, m] = 1.0
    return sel.reshape(16, NSEL * 128)


def _strip_self_waits(nc: bass.Bass) -> None:
    """Transitively-minimal semaphore waits (see module docstring)."""
    UPD = ("sem-inc", "sem-add-imm")
    insts = [i for bb in nc.m.functions[0].blocks for i in bb.instructions]

    bad_sems = set()
    for i in insts:
        si = getattr(i, "sync_info", None)
        if si is None:
            continue
        for u in si.on_update:
            if u.sync_type != "semaphore" or u.update_mode not in UPD:
                bad_sems.add(u.id)

    def fifo_of(i):
        si = i.sync_info
        eng = str(getattr(i, "engine", None))
        if type(i).__name__ == "InstDMACopy" and si is not None:
            for u in si.on_update:
                if u.sync_type == "semaphore" and u.update_mode in UPD:
                    return ("q", u.id)
        return ("e", eng)

    cum: dict = {}
    event: dict = {}
    fifo_pred: dict = {}
    last_in_fifo: dict = {}
    metas = []
    for idx, i in enumerate(insts):
        si = getattr(i, "sync_info", None)
        f = fifo_of(i)
        fifo_pred[idx] = last_in_fifo.get(f)
        last_in_fifo[f] = idx
        ups = []
        if si is not None:
            for u in si.on_update:
                if u.sync_type == "semaphore" and u.update_mode in UPD:
                    cum[u.id] = cum.get(u.id, 0) + u.update_value
                    event[(u.id, cum[u.id])] = idx
                    ups.append((u.id, cum[u.id]))
        metas.append((si, ups))

    def resolve(sem, k):
        v = k
        while (sem, v) not in event:
            v += 1
            if v > cum.get(sem, 0):
                return None
        return event[(sem, v)]

    cvc: list = [None] * len(insts)

    def get_cvc(idx):
        if cvc[idx] is not None:
            return cvc[idx]
        stack = [idx]
        while stack:
            j = stack[-1]
            if cvc[j] is not None:
                stack.pop()
                continue
            si, ups = metas[j]
            deps = []
            p = fifo_pred[j]
            if p is not None:
                deps.append(p)
            if si is not None:
                for w in si.on_wait:
                    if (
                        w.sync_type == "semaphore"
                        and w.wait_mode == "sem-ge-imm"
                        and w.id not in bad_sems
                    ):
                        e = resolve(w.id, w.wait_value)
                        if e is not None and e != j:
                            deps.append(e)
            pending = [d for d in deps if cvc[d] is None]
            if pending:
                stack.extend(pending)
                continue
            stack.pop()
            vc: dict = {}
            for d in deps:
                for s, v in cvc[d].items():
                    if vc.get(s, 0) < v:
                        vc[s] = v
            if si is not None:
                for w in si.on_wait:
                    if (
                        w.sync_type == "semaphore"
                        and w.wait_mode == "sem-ge-imm"
                        and w.id not in bad_sems
                    ):
                        if vc.get(w.id, 0) < w.wait_value:
                            vc[w.id] = w.wait_value
            for s, v in ups:
                if vc.get(s, 0) < v:
                    vc[s] = v
            cvc[j] = vc
        return cvc[idx]

    for idx, i in enumerate(insts):
        si, _ups = metas[idx]
        if si is None or not si.on_wait:
            continue
        base: dict = {}
        p = fifo_pred[idx]
        if p is not None:
            base = dict(get_cvc(p))
        sem_waits = [
            w
            for w in si.on_wait
            if w.sync_type == "semaphore"
            and w.wait_mode == "sem-ge-imm"
            and w.id not in bad_sems
        ]
        other = [w for w in si.on_wait if w not in sem_waits]

        def strength(w):
            e = resolve(w.id, w.wait_value)
            return len(get_cvc(e)) if e is not None else 0

        sem_waits.sort(key=strength, reverse=True)

        def wait_cvc(w):
            e = resolve(w.id, w.wait_value)
            vc = dict(get_cvc(e)) if e is not None else {}
            if vc.get(w.id, 0) < w.wait_value:
                vc[w.id] = w.wait_value
            return vc

        kept = sem_waits[:]
        changed = True
        while changed:
            changed = False
            for w in kept:
                cover = dict(base)
                for w2 in kept:
                    if w2 is w:
                        continue
                    for s, v in wait_cvc(w2).items():
                        if cover.get(s, 0) < v:
                            cover[s] = v
                if cover.get(w.id, 0) >= w.wait_value:
                    kept.remove(w)
                    changed = True
                    break
        if len(kept) + len(other) != len(si.on_wait):
            si.on_wait = other + kept


def _segs(g: int) -> list:
    # fbc sub-DMA / TT granularity per group: fine for the ramp group, big
    # (fewer DVE ops) mid-stream, fine again to shorten the pipeline tail
    if g == 0:
        return [7, 7, 7, 7]
    if g == GROUPS - 1:
        return [14, 7, 7]
    return [14, 14]


def _build_nc(strip: bool = True) -> bass.Bass:
    nc = bass.Bass()
    wt = nc.dram_tensor("wt", [128, NCHUNK * 128], BF16, kind="ExternalInput")
    gt2 = nc.dram_tensor("gt2", [128, BLOC * D], BF16, kind="ExternalInput")
    fbc = nc.dram_tensor("fbc", [128, GROUPS * NFBC * N], BF16,
                         kind="ExternalInput")
    ft = nc.dram_tensor("ft", [16, BLOC * D], BF16, kind="ExternalInput")
    sel = nc.dram_tensor("sel", [16, NSEL * 128], BF16, kind="ExternalInput")
    bias = nc.dram_tensor("bias", [128, 1], F32, kind="ExternalInput")
    out = nc.dram_tensor("out", [128, BLOC * D], BF16, kind="ExternalOutput")

    GSZ = NFBC * N              # fbc elems per group per partition
    AF = mybir.ActivationFunctionType

    with tile.TileContext(nc) as tc:
        with (
            tc.tile_pool(name="resident", bufs=1) as res,
            tc.tile_pool(name="fbcp", bufs=2) as fpool,
            tc.tile_pool(name="p", bufs=2) as ppool,
            tc.tile_pool(name="fsbp", bufs=2 * GROUPS) as spool,
            tc.tile_pool(name="osb", bufs=8) as opool,
            tc.tile_pool(name="acc", bufs=4, space="PSUM") as apool,
            tc.tile_pool(name="fps", bufs=2, space="PSUM") as qpool,
        ):
            gt2_sb = res.tile([128, BLOC * D], BF16)
            wt_sb = res.tile([128, NCHUNK * 128], BF16)
            ft_sb = res.tile([16, BLOC * D], BF16)
            sel_sb = res.tile([16, NSEL * 128], BF16)
            bias_sb = res.tile([128, 1], F32)
            flag = res.tile([128, 4], BF16)
            scr = res.tile([128, 1], BF16)

            # scalar-HWDGE ring: small/bulky resident inputs + outputs.
            # sync ring: group-0 gt2 slice, then the fbc stream (so the fbc
            # flood cannot delay the inputs' transfers and vice versa).
            nc.scalar.dma_start(out=bias_sb[:], in_=bias[:])
            nc.scalar.dma_start(out=ft_sb[:], in_=ft[:])
            nc.scalar.dma_start(out=sel_sb[:], in_=sel[:])
            nc.scalar.dma_start(out=wt_sb[:], in_=wt[:])
            nc.scalar.dma_start(out=gt2_sb[:, N:], in_=gt2[:, N:])
            nc.scalar.activation(bias_sb[0:1, 0:1], bias_sb[0:1, 0:1], AF.Copy)
            nc.sync.dma_start(out=gt2_sb[:, ts(0, N)], in_=gt2[:, ts(0, N)])
            nc.vector.tensor_copy(gt2_sb[0:1, 0:1], gt2_sb[0:1, 0:1])
            nc.vector.tensor_copy(ft_sb[0:1, 0:1], ft_sb[0:1, 0:1])
            nc.vector.tensor_copy(sel_sb[0:1, 0:1], sel_sb[0:1, 0:1])
            nc.vector.tensor_copy(wt_sb[0:1, 0:1], wt_sb[0:1, 0:1])
            nc.vector.tensor_copy(gt2_sb[0:1, N:N + 1], gt2_sb[0:1, N:N + 1])

            fbc_tiles = {}

            def issue_fbc(g):
                t = fpool.tile([128, NFBC, N], BF16, tag="fbc")
                fbc_tiles[g] = t
                c0 = 0
                for sg in _segs(g):
                    nc.sync.dma_start(
                        out=t[:, c0:c0 + sg, :],
                        in_=fbc[:, g * GSZ + c0 * N:g * GSZ + (c0 + sg) * N],
                    )
                    c0 += sg

            def sel_mms(g):
                # selector matmuls producing group g's on-chip broadcast
                # pairs (chunks NFBC..31) into PSUM, K=16 compact contraction
                tiles = []
                for u in range(NSEL // 2):
                    fp = qpool.tile([128, 2, N], F32, tag="fps")
                    for d_ in range(2):
                        cs = 2 * u + d_
                        nc.tensor.matmul(fp[:, d_, :],
                                         sel_sb[:, ts(cs, 128)],
                                         ft_sb[:, ts(g, N)],
                                         start=True, stop=True)
                    tiles.append(fp)
                return tiles

            def act_copies(fps_tiles, g):
                # fsb pool has one buffer per (group, pair): never reused, so
                # these ACT copies carry no WAR wait (1-wait struct limit)
                tiles = []
                for u, fp in enumerate(fps_tiles):
                    fsb = spool.tile([128, 2, N], BF16, tag="fsb")
                    nc.scalar.activation(fsb[:], fp[:], AF.Copy)
                    tiles.append(fsb)
                return tiles

            issue_fbc(0)
            issue_fbc(1)
            fps_cur = sel_mms(0)
            fsb_cur = act_copies(fps_cur, 0)

            for g in range(GROUPS):
                fbc_t = fbc_tiles.pop(g)
                if g + 2 < GROUPS:
                    issue_fbc(g + 2)
                acc = apool.tile([128, N], F32, tag="acc")
                gblk = gt2_sb[:, ts(g, N)]
                # deferred flag-read: pulls group g-2's ACT completion onto
                # the DVE clock well before group g+2 needs it for acc reuse,
                # without stalling DVE at the g-1/g boundary (identity(g-2)
                # finished long ago).  4 flag slots keep the RAW dep bound to
                # the right flag-copy.
                if g >= 2:
                    s_ = (g - 2) % 4
                    nc.vector.tensor_copy(scr[0:1, 0:1],
                                          flag[0:1, s_:s_ + 1])
                c0 = 0
                for si_, sg in enumerate(_segs(g)):
                    # touch: each sub-DMA's completion sem onto the DVE clock
                    nc.vector.tensor_copy(fbc_t[0:1, c0, 0:1],
                                          fbc_t[0:1, c0, 0:1])
                    p = ppool.tile([128, sg, N], BF16, tag=f"p{sg}", bufs=2)
                    gview = gblk[:, None, :].to_broadcast((128, sg, N))
                    nc.vector.tensor_mul(p[:], gview,
                                         fbc_t[:, c0:c0 + sg, :])
                    for u in range(sg):
                        c = c0 + u
                        nc.tensor.matmul(acc[:], wt_sb[:, ts(c, 128)],
                                         p[:, u, :], start=(c == 0),
                                         stop=(c == NFBC - 1))
                    c0 += sg

                    if si_ == 0:
                        # on-chip chunks NFBC..31 run mid-group, right after
                        # the first main segment, so their ACT->DVE->PE chain
                        # (identity(g-1) -> copies(g) -> TT -> MMs) stays off
                        # the accumulation's critical tail.  pp tiles are
                        # never reused -> the TTs carry only their single
                        # fsb-RAW (ACT) wait.
                        for u, fsb in enumerate(fsb_cur):
                            p2 = ppool.tile([128, 2, N], BF16, tag="pp",
                                            bufs=GROUPS * NSEL // 2)
                            gview = gblk[:, None, :].to_broadcast((128, 2, N))
                            nc.vector.tensor_mul(p2[:], gview, fsb[:])
                            for d_ in range(2):
                                c = NFBC + 2 * u + d_
                                nc.tensor.matmul(acc[:], wt_sb[:, ts(c, 128)],
                                                 p2[:, d_, :], start=False,
                                                 stop=False)

                # next group's selector pipeline (PE then ACT), issued at the
                # end of this group's PE stream so WAR chains stay implied
                if g + 1 < GROUPS:
                    fps_cur = sel_mms(g + 1)

                osb = opool.tile([128, N], BF16, tag="osb")
                nc.scalar.activation(osb[:], acc[:], AF.Identity,
                                     bias=bias_sb[:, 0:1], scale=1.0)
                if g + 1 < GROUPS:
                    fsb_cur = act_copies(fps_cur, g + 1)
                # identity(g) -> flag-copy (reads osb!): REAL data deps (the
                # scheduler reorders freely; only data deps pin order); the
                # matching DVE read happens at the start of group g+2
                s_ = g % 4
                nc.scalar.activation(flag[0:1, s_:s_ + 1], osb[0:1, 1:2],
                                     AF.Copy)
                nc.scalar.dma_start(out=out[:, ts(g, N)], in_=osb[:])
                # WAR consumer: pulls the out-DMA's queue-sem onto the ACT
                # clock so the kernel-tail drain needs only engine waits
                nc.scalar.activation(osb[0:1, 0:1], osb[0:1, 0:1], AF.Copy)
                last_osb = osb

            # fold the ACT-final clock onto DVE so the kernel-exit Drain
            # (a 1-wait CTRL struct) needs only the DVE sem
            nc.vector.tensor_copy(scr[0:1, 0:1], last_osb[0:1, 0:1])

    if strip:
        _strip_self_waits(nc)
    return nc


def _get_nc() -> bass.Bass:
    if "nc" not in _CACHE:
        _CACHE["nc"] = _build_nc()
    return _CACHE["nc"]


def _prep_core_inputs(feature, base, W, b, ci):
    bf16 = mybir.dt.np(BF16)
    bsl = slice(ci * BLOC, (ci + 1) * BLOC)
    F = np.ascontiguousarray(feature[bsl], np.float32)  # (128, 64, 32)
    G = np.ascontiguousarray(base[bsl], np.float32)     # (128, 64, 32)

    Gt = np.transpose(G, (1, 0, 2))                      # (j, b, d)
    gt2 = np.concatenate([Gt, Gt], 0).reshape(128, BLOC * D)

    # wt[p, c, o] = W[o, 128c + p]
    wt = np.transpose(W.reshape(O, NCHUNK, 128), (2, 1, 0)).reshape(
        128, NCHUNK * 128)

    # fbc[p=(dup,64), g, c, n] = F-transposed[i=2c+dup, g, n] replicated over
    # the 64 j-partitions of each dup-half; chunks 0..25 only (26..31 are
    # produced on-chip from ft via selector matmuls)
    Ft = np.transpose(F, (1, 0, 2)).reshape(HK, GROUPS, N)   # [i, g, n]
    A = Ft.reshape(NCHUNK, 2, GROUPS, N).astype(bf16)        # [c, dup, g, n]
    fbc = np.broadcast_to(
        A.transpose(1, 2, 0, 3)[:, None, :, :NFBC], (2, 64, GROUPS, NFBC, N)
    )
    fbc = np.ascontiguousarray(fbc).reshape(128, GROUPS * NFBC * N)

    # compact ft rows: F rows 2*NFBC.. at partitions 0.., zero-padded to 16
    nr = 2 * NSEL
    ftc = np.zeros((16, BLOC * D), bf16)
    ftc[:nr] = np.transpose(F[:, 2 * NFBC:, :], (1, 0, 2)).reshape(
        nr, BLOC * D)

    return {
        "wt": np.ascontiguousarray(wt.astype(bf16)),
        "gt2": np.ascontiguousarray(gt2.astype(bf16)),
        "fbc": fbc,
        "ft": ftc,
        "sel": np.ascontiguousarray(_sel_const().astype(bf16)),
        "bias": np.ascontiguousarray(b, np.float32).reshape(128, 1),
    }


def run(feature, base, W, b, **spmd_kwargs):
    nc = _get_nc()
    in_maps = [_prep_core_inputs(feature, base, W, b, ci) for ci in range(NCORES)]
    res = run_bass_kernel_spmd(nc, in_maps, list(range(NCORES)), **spmd_kwargs)
    outs = []
    for ci in range(NCORES):
        o = res.results[ci]["out"].astype(np.float32).reshape(O, BLOC, D)
        outs.append(np.transpose(o, (1, 0, 2)))
    full = np.concatenate(outs, 0)
    return full, res


def kernel(feature, base, W, b):
    full, _ = run(feature, base, W, b)
    return full
